# revision 10
# baseline (speedup 1.0000x reference)
"""GCC-PHAT kernel for Trainium2, 8 NeuronCores, data-parallel over batch.

Input : x [128, 12, 4096] f32
Output: [128, 12, 12, 257] f32

Per core (16 batches):
  rfft(4096) via 2-stage Cooley-Tukey (32 x 128) on the tensor engine,
  PHAT normalize (ACT+DVE), pairwise cross-power for the 66 unordered
  pairs (DVE), lag-restricted inverse DFT as matmul with even/odd (E/O)
  lag splitting, diagonal pairs and (m,n) mirrors filled on host.

Self-contained: hardcodes shapes; only needs /opt/trn_rl_repo on sys.path.
"""
import os
import sys

sys.path.insert(0, "/opt/trn_rl_repo")

import numpy as np

B = 16            # batches per core
NSIG = 12
K = 4096
TAU = 128
NCORES = 8
NS = B * NSIG     # 192 signals per core
NPAIR = NSIG * (NSIG - 1) // 2   # 66
ROWS = B * NPAIR  # 1056
PAIRS = [(n, m) for n in range(NSIG) for m in range(n + 1, NSIG)]
POFF = {}
_off = 0
for n in range(NSIG):
    POFF[n] = _off
    _off += NSIG - 1 - n

# ---- precision config ----
DT_A = "float32"     # corner-turned A + stage2 weights (matmul dtype of stage 2)
DT_X = "float32"     # PHAT-normalized spectrum storage
DT_G = "float32"     # cross-power + inverse matmul dtype
FWD_F32R = False     # stage-1 matmul as float32r (full rate, N=512)
HALVES = 2 if DT_G == "float32" else 1   # split rows for SBUF residency
ROWSH = ROWS // HALVES                    # 528 (or 1056)

_COMPILED = {}


def _dt(name):
    from concourse import mybir
    return getattr(mybir.dt, name)


def _npdt(name):
    import ml_dtypes
    return {"float32": np.float32, "bfloat16": ml_dtypes.bfloat16}[name]


def _build_weights():
    """All weights in exact device SBUF layouts."""
    f32 = np.float32
    n1 = np.arange(32)[:, None]
    q = np.arange(32)[None, :]
    ang1 = 2 * np.pi * n1 * q / 32.0
    w1_single = np.concatenate([np.cos(ang1), -np.sin(ang1)], axis=1)  # [32,64]
    w1 = np.concatenate([w1_single, w1_single], axis=0).astype(f32)    # [64,64]

    ident = np.eye(128, dtype=f32)

    # stage2: w2d [128 n2, (q 32, t 3, k2 64)] ; t: 0=re, 1=-im, 2=+im
    n2 = np.arange(128)[:, None]
    k2 = np.arange(64)[None, :]
    w2 = np.zeros((128, 32, 3, 64), dtype=np.float64)
    for qv in range(32):
        ang = 2 * np.pi * (qv * n2 / 4096.0 + n2 * k2 / 128.0)
        w2[:, qv, 0, :] = np.cos(ang)
        w2[:, qv, 1, :] = np.sin(ang)    # -(-sin) = +sin  (this is -w2im)
        w2[:, qv, 2, :] = -np.sin(ang)   # w2im
    w2d = w2.reshape(128, 32 * 3 * 64).astype(_npdt(DT_A))

    wnyq = ((-1.0) ** np.arange(128)).reshape(128, 1).astype(f32)

    # inverse weights, chunk order p=(k2 | k2'), j -> f = q + 32*k2
    p = np.arange(128)
    jj = np.arange(16)[:, None]
    qq = np.where(p[None, :] < 64, 2 * jj, 2 * jj + 1)
    kk2 = np.where(p[None, :] < 64, p[None, :], p[None, :] - 64)
    fmap = qq + 32 * kk2                               # [16,128]
    cf = np.where(fmap == 0, 1.0, 2.0) / K
    l = np.arange(1, 129)[None, None, :]
    ang = 2 * np.pi * fmap[:, :, None] * l / K
    cmat = cf[:, :, None] * np.cos(ang)                # [16,128,128] (j, p, l)
    smat = -cf[:, :, None] * np.sin(ang)
    # device layout [128 p, (16 j, 128 l)]
    cmatd = cmat.transpose(1, 0, 2).reshape(128, 16 * 128).astype(_npdt(DT_G))
    smatd = smat.transpose(1, 0, 2).reshape(128, 16 * 128).astype(_npdt(DT_G))
    c0d = cf.T.astype(_npdt(DT_G)).copy()              # [128 p, 16 j]
    cnd = ((1.0 / K) * ((-1.0) ** np.arange(1, 129))).reshape(1, 128).astype(_npdt(DT_G))
    onekd = np.full((1, 1), 1.0 / K, dtype=_npdt(DT_G))
    return dict(w1d=w1, identd=ident, w2d=w2d, wnyqd=wnyq,
                cmatd=cmatd, smatd=smatd, c0d=c0d, cnd=cnd, onekd=onekd)


def _legalize_waits(nc):
    """This container's walrus accepts only ONE sync-wait per instruction.
    Split extra waits into single-wait NoOps inserted before, same engine."""
    from concourse import mybir
    nsplit = 0
    for b in nc.main_func.blocks:
        newlist = []
        for ins in b.instructions:
            si = ins.sync_info
            if si is not None and len(si.on_wait) > 1:
                waits = list(si.on_wait)
                for k, wt in enumerate(waits[:-1]):
                    nop = mybir.InstNoOp(name=f"{ins.name}-lw{k}", ins=[], outs=[])
                    nop.engine = ins.engine
                    nop.sync_info = mybir.SyncInfo(on_wait=[wt], on_update=[])
                    newlist.append(nop)
                    nsplit += 1
                ins.sync_info = mybir.SyncInfo(on_wait=[waits[-1]],
                                               on_update=list(si.on_update))
            newlist.append(ins)
        b.instructions = newlist
    return nsplit


def _build_bass():
    from concourse import bass, mybir, tile

    f32 = mybir.dt.float32
    dtA, dtX, dtG = _dt(DT_A), _dt(DT_X), _dt(DT_G)
    AF = mybir.ActivationFunctionType

    nc = bass.Bass()
    xd = nc.declare_dram_parameter("x", [NS, 32, 128], f32, isOutput=False)
    w1d = nc.declare_dram_parameter("w1d", [64, 64], f32, isOutput=False)
    identd = nc.declare_dram_parameter("identd", [128, 128], f32, isOutput=False)
    w2d = nc.declare_dram_parameter("w2d", [128, 32 * 3 * 64], dtA, isOutput=False)
    wnyqd = nc.declare_dram_parameter("wnyqd", [128, 1], f32, isOutput=False)
    cmatd = nc.declare_dram_parameter("cmatd", [128, 16 * 128], dtG, isOutput=False)
    smatd = nc.declare_dram_parameter("smatd", [128, 16 * 128], dtG, isOutput=False)
    c0d = nc.declare_dram_parameter("c0d", [128, 16], dtG, isOutput=False)
    cnd = nc.declare_dram_parameter("cnd", [1, 128], dtG, isOutput=False)
    onekd = nc.declare_dram_parameter("onekd", [1, 1], dtG, isOutput=False)

    outpd = nc.declare_dram_parameter("outp", [128, ROWS], f32, isOutput=True)
    outmd = nc.declare_dram_parameter("outm", [128, ROWS], f32, isOutput=True)
    out0d = nc.declare_dram_parameter("out0", [128, 2 * ((ROWSH + 127) // 128)], f32,
                                      isOutput=True)

    NBLK = (ROWSH + 127) // 128  # lag-0 col chunks per half

    with tile.TileContext(nc) as tc:
        with (
            tc.tile_pool(name="const", bufs=1) as cpool,
            tc.tile_pool(name="big", bufs=1) as bigp,
        ):
            # --- load constants ---
            w1sb = cpool.tile([64, 64], f32, tag="w1sb")
            nc.sync.dma_start(out=w1sb[:], in_=w1d[:])
            identsb = cpool.tile([128, 128], f32, tag="identsb")
            nc.sync.dma_start(out=identsb[:], in_=identd[:])
            wnyqsb = cpool.tile([128, 1], f32, tag="wnyqsb")
            nc.sync.dma_start(out=wnyqsb[:], in_=wnyqd[:])
            cmatsb = cpool.tile([128, 2048], dtG, tag="cmatsb")
            nc.sync.dma_start(out=cmatsb[:], in_=cmatd[:])
            smatsb = cpool.tile([128, 2048], dtG, tag="smatsb")
            nc.sync.dma_start(out=smatsb[:], in_=smatd[:])
            c0sb = cpool.tile([128, 16], dtG, tag="c0sb")
            nc.sync.dma_start(out=c0sb[:], in_=c0d[:])
            cnsb = cpool.tile([1, 128], dtG, tag="cnsb")
            nc.sync.dma_start(out=cnsb[:], in_=cnd[:])
            oneksb = cpool.tile([1, 1], dtG, tag="oneksb")
            nc.sync.dma_start(out=oneksb[:], in_=onekd[:])

            Xre = bigp.tile([128, 16 * NS], dtX, tag="Xre")
            Xim = bigp.tile([128, 16 * NS], dtX, tag="Xim")

            xnyqsb = cpool.tile([1, NS], f32, tag="xnyqsb")
            snyq = cpool.tile([1, NS], f32, tag="snyq")
            g2048 = cpool.tile([1, ROWS], dtG, tag="g2048")

            outpsb = cpool.tile([128, ROWS], f32, tag="outpsb")
            outmsb = cpool.tile([128, ROWS], f32, tag="outmsb")
            out0sb = cpool.tile([128, 2 * NBLK], f32, tag="out0sb")

            # x DRAM view: sig s = 8t + 2j + g -> [t, g, n1, j, n2]
            xview = xd[:].rearrange("(t j g) a b -> t g a j b", t=24, j=4, g=2)

            fwd_scope = tc.tile_pool(name="fwd", bufs=1)
            fwdp = fwd_scope.__enter__()
            xin_scope = tc.tile_pool(name="xin", bufs=3)
            xinp = xin_scope.__enter__()
            s1_scope = tc.tile_pool(name="s1sb", bufs=3)
            s1p = s1_scope.__enter__()

            AT = fwdp.tile([128, 24 * 512], dtA, tag="AT")
            atv = AT[:].rearrange("p (t j g r q) -> p t j g r q",
                                  t=24, j=4, g=2, r=2, q=32)
            w2sb = fwdp.tile([128, 32 * 3 * 64], dtA, tag="w2sb")
            nc.sync.dma_start(out=w2sb[:], in_=w2d[:])

            # ---------- phase A: stage 1 + corner turn ----------
            with tc.tile_pool(name="psA", bufs=2, space="PSUM") as psA:
                for t in range(24):
                    xt = xinp.tile([64, 512], f32, tag="xt")
                    nc.sync.dma_start(out=xt[:], in_=xview[t])
                    ps = psA.tile([128, 512], f32, tag="s1")
                    if FWD_F32R:
                        r32 = mybir.dt.float32r
                        nc.tensor.matmul(ps[0:64, :], w1sb[0:32, :].bitcast(r32),
                                         xt[0:32, :].bitcast(r32), start=True, stop=True)
                        nc.tensor.matmul(ps[64:128, :], w1sb[32:64, :].bitcast(r32),
                                         xt[32:64, :].bitcast(r32), start=True, stop=True)
                    else:
                        nc.tensor.matmul(ps[0:64, :], w1sb[0:32, :], xt[0:32, :],
                                         start=True, stop=True)
                        nc.tensor.matmul(ps[64:128, :], w1sb[32:64, :], xt[32:64, :],
                                         start=True, stop=True)
                    s1s = s1p.tile([128, 512], f32, tag="s1s")
                    if t % 2 == 0:
                        nc.scalar.copy(s1s[:], ps[:])
                    else:
                        nc.vector.tensor_copy(s1s[:], ps[:])
                    tp = psA.tile([128, 512], f32, tag="tp")
                    for b4 in range(4):
                        nc.tensor.transpose(tp[:, 128 * b4:128 * (b4 + 1)],
                                            s1s[:, 128 * b4:128 * (b4 + 1)],
                                            identsb[:])
                    dst = AT[:, 512 * t:512 * (t + 1)]
                    if t % 2 == 0:
                        nc.vector.tensor_copy(dst, tp[:])
                    else:
                        nc.scalar.copy(dst, tp[:])

                # nyquist: X[2048] = sum_n2 (-1)^n2 * Are[q=0]
                are0 = atv[:, :, :, :, 0, 0]
                psn = psA.tile([1, NS], f32, tag="xnyq")
                nc.tensor.matmul(psn[:], wnyqsb[:], are0, start=True, stop=True)
                nc.scalar.copy(xnyqsb[:], psn[:])

            # ---------- phase B: stage 2 ----------
            w2v = w2sb[:].rearrange("p (q t k) -> p q t k", q=32, t=3, k=64)
            with tc.tile_pool(name="psB", bufs=2, space="PSUM") as psB:
                for jq in range(16):
                    x2 = psB.tile([128, 384], f32, tag="x2")
                    for par in range(2):
                        qv = 2 * jq + par
                        are = atv[:, :, :, :, 0, qv]
                        aim = atv[:, :, :, :, 1, qv]
                        if DT_A == "float32r_never":
                            pass
                        re_out = x2[64 * par:64 * (par + 1), 0:192]
                        im_out = x2[64 * par:64 * (par + 1), 192:384]
                        nc.tensor.matmul(re_out, w2v[:, qv, 0, :], are,
                                         start=True, stop=False)
                        nc.tensor.matmul(re_out, w2v[:, qv, 1, :], aim,
                                         start=False, stop=True)
                        nc.tensor.matmul(im_out, w2v[:, qv, 2, :], are,
                                         start=True, stop=False)
                        nc.tensor.matmul(im_out, w2v[:, qv, 0, :], aim,
                                         start=False, stop=True)
                    nc.scalar.copy(Xre[:, NS * jq:NS * (jq + 1)], x2[:, 0:192])
                    nc.vector.tensor_copy(Xim[:, NS * jq:NS * (jq + 1)], x2[:, 192:384])

            # ---------- PHAT ----------
            t1 = fwdp.tile([128, 16 * NS], f32, tag="t1")
            t2 = fwdp.tile([128, 16 * NS], f32, tag="t2")
            nc.scalar.activation(t1[:], Xre[:], AF.Square)
            nc.scalar.activation(t2[:], Xim[:], AF.Square)
            nc.vector.tensor_add(t1[:], t1[:], t2[:])
            nc.scalar.activation(t2[:], t1[:], AF.Ln)
            nc.scalar.activation(t1[:], t2[:], AF.Exp, scale=-0.5)
            nc.vector.tensor_mul(Xre[:], Xre[:], t1[:])
            nc.vector.tensor_mul(Xim[:], Xim[:], t1[:])
            nc.scalar.sign(snyq[:], xnyqsb[:])

            # nyquist pair row
            snv = snyq[:].rearrange("p (b n) -> p b n", b=B, n=NSIG)
            g2v = g2048[:].rearrange("p (b r) -> p b r", b=B, r=NPAIR)
            for n in range(NSIG - 1):
                mc = NSIG - 1 - n
                an = snv[:, :, n].unsqueeze(2).broadcast_to((1, B, mc))
                am = snv[:, :, n + 1:]
                nc.vector.tensor_mul(g2v[:, :, POFF[n]:POFF[n] + mc], an, am)

            # ---------- cross-power + inverse, per half ----------
            s1_scope.__exit__(None, None, None)
            xin_scope.__exit__(None, None, None)
            fwd_scope.__exit__(None, None, None)
            xrev = Xre[:].rearrange("p (j b n) -> p j b n", j=16, b=B, n=NSIG)
            ximv = Xim[:].rearrange("p (j b n) -> p j b n", j=16, b=B, n=NSIG)
            BH = B // HALVES
            with (
                tc.tile_pool(name="gpool", bufs=1) as gp,
                tc.tile_pool(name="tmpp", bufs=1) as tmpp,
                tc.tile_pool(name="psC", bufs=1, space="PSUM") as psC,
                tc.tile_pool(name="ps0", bufs=2, space="PSUM") as ps0,
            ):
                for h in range(HALVES):
                    b0 = h * BH
                    Gre = gp.tile([128, 16 * ROWSH], dtG, tag="Gre")
                    Gim = gp.tile([128, 16 * ROWSH], dtG, tag="Gim")
                    grev = Gre[:].rearrange("p (j b r) -> p j b r", j=16, b=BH, r=NPAIR)
                    gimv = Gim[:].rearrange("p (j b r) -> p j b r", j=16, b=BH, r=NPAIR)
                    tt1 = tmpp.tile([128, 16 * BH * (NSIG - 1)], dtG, tag="tt1")
                    tt2 = tmpp.tile([128, 16 * BH * (NSIG - 1)], dtG, tag="tt2")
                    t1v = tt1[:].rearrange("p (j b m) -> p j b m", j=16, b=BH, m=NSIG - 1)
                    t2v = tt2[:].rearrange("p (j b m) -> p j b m", j=16, b=BH, m=NSIG - 1)
                    for n in range(NSIG - 1):
                        mc = NSIG - 1 - n
                        an = xrev[:, :, b0:b0 + BH, n].unsqueeze(3).broadcast_to(
                            (128, 16, BH, mc))
                        bn = ximv[:, :, b0:b0 + BH, n].unsqueeze(3).broadcast_to(
                            (128, 16, BH, mc))
                        am = xrev[:, :, b0:b0 + BH, n + 1:]
                        bm = ximv[:, :, b0:b0 + BH, n + 1:]
                        o_re = grev[:, :, :, POFF[n]:POFF[n] + mc]
                        o_im = gimv[:, :, :, POFF[n]:POFF[n] + mc]
                        u1 = t1v[:, :, :, 0:mc]
                        u2 = t2v[:, :, :, 0:mc]
                        nc.vector.tensor_mul(u1, an, am)
                        nc.vector.tensor_mul(u2, bn, bm)
                        nc.vector.tensor_add(o_re, u1, u2)
                        nc.vector.tensor_mul(u1, bn, am)
                        nc.vector.tensor_mul(u2, an, bm)
                        nc.vector.tensor_sub(o_im, u1, u2)

                    # inverse: E/O over row chunks
                    grev2 = Gre[:].rearrange("p (j r) -> p j r", j=16, r=ROWSH)
                    gimv2 = Gim[:].rearrange("p (j r) -> p j r", j=16, r=ROWSH)
                    rcs = []
                    rc0 = 0
                    while rc0 < ROWSH:
                        rcs.append((rc0, min(512, ROWSH - rc0)))
                        rc0 += 512
                    for ci, (c0, cw) in enumerate(rcs):
                        gc = h * ROWSH + c0
                        eps = psC.tile([128, cw], f32, tag=f"e{ci}")
                        ops = psC.tile([128, cw], f32, tag=f"o{ci}")
                        for jq in range(16):
                            nc.tensor.matmul(eps[:], cmatsb[:, 128 * jq:128 * (jq + 1)],
                                             grev2[:, jq, c0:c0 + cw],
                                             start=(jq == 0), stop=False)
                        nc.tensor.matmul(eps[:], cnsb[:], g2048[:, gc:gc + cw],
                                         start=False, stop=True)
                        for jq in range(16):
                            nc.tensor.matmul(ops[:], smatsb[:, 128 * jq:128 * (jq + 1)],
                                             gimv2[:, jq, c0:c0 + cw],
                                             start=(jq == 0), stop=(jq == 15))
                        esb = tmpp.tile([128, 512], f32, tag="esb")
                        nc.scalar.copy(esb[0:128, 0:cw], eps[:])
                        nc.vector.tensor_add(outpsb[:, gc:gc + cw],
                                             esb[0:128, 0:cw], ops[:])
                        nc.vector.tensor_sub(outmsb[:, gc:gc + cw],
                                             esb[0:128, 0:cw], ops[:])
                        nc.sync.dma_start(out=outpd[:, gc:gc + cw],
                                          in_=outpsb[:, gc:gc + cw])
                        nc.sync.dma_start(out=outmd[:, gc:gc + cw],
                                          in_=outmsb[:, gc:gc + cw])

                    # lag-0 side channel
                    for bi in range(NBLK):
                        c0 = 128 * bi
                        cw = min(128, ROWSH - c0)
                        gc = h * ROWSH + c0
                        z = ps0.tile([128, 1], f32, tag="z")
                        for jq in range(16):
                            nc.tensor.matmul(z[0:cw, :], grev2[:, jq, c0:c0 + cw],
                                             c0sb[:, jq:jq + 1],
                                             start=(jq == 0), stop=False)
                        nc.tensor.matmul(z[0:cw, :], g2048[:, gc:gc + cw],
                                         oneksb[:], start=False, stop=True)
                        nc.scalar.copy(out0sb[0:cw, h * NBLK + bi:h * NBLK + bi + 1],
                                       z[0:cw, :])
            nc.sync.dma_start(out=out0d[:], in_=out0sb[:])

    _legalize_waits(nc)
    return nc


def _get_compiled():
    if "nc" not in _COMPILED:
        _COMPILED["nc"] = _build_bass()
        _COMPILED["weights"] = _build_weights()
    return _COMPILED["nc"], _COMPILED["weights"]


def kernel(x: np.ndarray) -> np.ndarray:
    from concourse.bass_utils import run_bass_kernel_spmd

    nc, W = _get_compiled()
    x = np.ascontiguousarray(x, dtype=np.float32)
    in_maps = []
    for c in range(NCORES):
        xc = x[c * B:(c + 1) * B].reshape(NS, 32, 128)
        m = {"x": np.ascontiguousarray(xc)}
        m.update(W)
        in_maps.append(m)

    trace = bool(int(os.environ.get("BASS_GCC_TRACE", "0")))
    res = run_bass_kernel_spmd(nc, in_maps, list(range(NCORES)), trace=trace)
    _COMPILED["last_result"] = res

    NBLK = (ROWSH + 127) // 128
    out = np.zeros((NCORES * B, NSIG, NSIG, 2 * TAU + 1), dtype=np.float32)
    for c in range(NCORES):
        r = res.results[c]
        outp, outm, out0 = r["outp"], r["outm"], r["out0"]
        # lag-0 per row
        z = np.zeros(ROWS, np.float32)
        for h in range(HALVES):
            for bi in range(NBLK):
                c0 = 128 * bi
                cw = min(128, ROWSH - c0)
                z[h * ROWSH + c0:h * ROWSH + c0 + cw] = out0[0:cw, h * NBLK + bi]
        for pi, (n, m) in enumerate(PAIRS):
            rows = np.arange(B) * NPAIR + pi
            blk = out[c * B:(c + 1) * B]
            blk[:, n, m, 0] = z[rows]
            blk[:, n, m, 1:129] = outp[:, rows].T
            blk[:, n, m, 129:] = outm[::-1, rows].T
            blk[:, m, n, 0] = z[rows]
            blk[:, m, n, 1:] = blk[:, n, m, 1:][:, ::-1]
        for n in range(NSIG):
            out[c * B:(c + 1) * B, n, n, 0] = 1.0
    return out


# revision 48
# speedup vs baseline: 331.7255x; 331.7255x over previous
"""GCC-PHAT kernel for Trainium2, 8 NeuronCores, data-parallel over batch.

Input : x [128, 12, 4096] f32
Output: [128, 12, 12, 257] f32

Per core (16 batches):
  rfft(4096) via 2-stage Cooley-Tukey (32 x 128) on the tensor engine,
  PHAT normalize (ACT+DVE), pairwise cross-power for the 66 unordered
  pairs (DVE), lag-restricted inverse DFT as matmul with even/odd (E/O)
  lag splitting, diagonal pairs and (m,n) mirrors filled on host.

Self-contained: hardcodes shapes; only needs /opt/trn_rl_repo on sys.path.
"""
import os
import sys

sys.path.insert(0, "/opt/trn_rl_repo")

import numpy as np

B = 16            # batches per core
NSIG = 12
K = 4096
TAU = 128
NCORES = 8
NS = B * NSIG     # 192 signals per core
NPAIR = NSIG * (NSIG - 1) // 2   # 66
ROWS = B * NPAIR  # 1056
PAIRS = [(n, m) for n in range(NSIG) for m in range(n + 1, NSIG)]
POFF = {}
_off = 0
for n in range(NSIG):
    POFF[n] = _off
    _off += NSIG - 1 - n

# ---- precision config ----
DT_A = "float16"     # corner-turned A + stage2 weights (matmul dtype of stage 2)
DT_X = "float16"     # PHAT-normalized spectrum storage
DT_G = "float16"     # cross-power + inverse matmul dtype
FWD_F32R = False     # stage-1 matmul dtype is fp32 (x stays exact)
HALVES = 2 if DT_G == "float32" else 1   # split rows for SBUF residency
ROWSH = ROWS // HALVES                    # 528 (or 1056)

_COMPILED = {}


def _dt(name):
    from concourse import mybir
    return getattr(mybir.dt, name)


def _npdt(name):
    import ml_dtypes
    return {"float32": np.float32, "bfloat16": ml_dtypes.bfloat16,
            "float16": np.float16}[name]


def _build_weights():
    """All weights in exact device SBUF layouts."""
    f32 = np.float32
    n1 = np.arange(32)[:, None]
    q = np.arange(32)[None, :]
    ang1 = 2 * np.pi * n1 * q / 32.0
    w1_single = np.concatenate([np.cos(ang1), -np.sin(ang1)], axis=1)  # [32,64]
    # block-diagonal [64, 128]: two independent 32->64 DFTs in one matmul
    w1 = np.zeros((64, 128))
    w1[0:32, 0:64] = w1_single
    w1[32:64, 64:128] = w1_single
    w1 = w1.astype(_npdt(DT_A))

    ident = np.eye(128).astype(_npdt(DT_A))

    # stage2: w2d [128 n2, (q 32, t 3, k2 64)] ; t: 0=re, 1=-im, 2=+im
    n2 = np.arange(128)[:, None]
    k2 = np.arange(64)[None, :]
    w2 = np.zeros((128, 32, 3, 64), dtype=np.float64)
    for qv in range(32):
        ang = 2 * np.pi * (qv * n2 / 4096.0 + n2 * k2 / 128.0)
        w2[:, qv, 0, :] = np.cos(ang)
        w2[:, qv, 1, :] = np.sin(ang)    # -(-sin) = +sin  (this is -w2im)
        w2[:, qv, 2, :] = -np.sin(ang)   # w2im
    w2d = w2.reshape(128, 32 * 3 * 64).astype(_npdt(DT_A))

    wnyq = ((-1.0) ** np.arange(128)).reshape(128, 1).astype(_npdt(DT_A))

    # inverse weights, chunk order p=(k2 | k2'), j -> f = q + 32*k2
    p = np.arange(128)
    jj = np.arange(16)[:, None]
    qq = np.where(p[None, :] < 64, 2 * jj, 2 * jj + 1)
    kk2 = np.where(p[None, :] < 64, p[None, :], p[None, :] - 64)
    fmap = qq + 32 * kk2                               # [16,128]
    cf = np.where(fmap == 0, 1.0, 2.0) / K
    l = np.arange(1, 129)[None, None, :]
    ang = 2 * np.pi * fmap[:, :, None] * l / K
    cmat = cf[:, :, None] * np.cos(ang)                # [16,128,128] (j, p, l)
    smat = -cf[:, :, None] * np.sin(ang)
    # device layout [128 p, (16 j, 128 l)]
    cmatd = cmat.transpose(1, 0, 2).reshape(128, 16 * 128).astype(_npdt(DT_G))
    smatd = smat.transpose(1, 0, 2).reshape(128, 16 * 128).astype(_npdt(DT_G))
    c0d = cf.T.astype(_npdt(DT_G)).copy()              # [128 p, 16 j]
    cnd = ((1.0 / K) * ((-1.0) ** np.arange(1, 129))).reshape(1, 128).astype(_npdt(DT_G))
    onekd = np.full((1, 1), 1.0 / K, dtype=_npdt(DT_G))
    return dict(w1d=w1, identd=ident, w2d=w2d, wnyqd=wnyq,
                cmatd=cmatd, smatd=smatd, c0d=c0d, cnd=cnd, onekd=onekd)


def _legalize_waits(nc):
    """This container's walrus accepts only ONE sync-wait per instruction.
    Split extra waits into single-wait NoOps inserted before, same engine."""
    from concourse import mybir
    nsplit = 0
    for b in nc.main_func.blocks:
        newlist = []
        for ins in b.instructions:
            si = ins.sync_info
            if si is not None and len(si.on_wait) > 1:
                waits = list(si.on_wait)
                for k, wt in enumerate(waits[:-1]):
                    nop = mybir.InstNoOp(name=f"{ins.name}-lw{k}", ins=[], outs=[])
                    nop.engine = ins.engine
                    nop.sync_info = mybir.SyncInfo(on_wait=[wt], on_update=[])
                    newlist.append(nop)
                    nsplit += 1
                ins.sync_info = mybir.SyncInfo(on_wait=[waits[-1]],
                                               on_update=list(si.on_update))
            newlist.append(ins)
        b.instructions = newlist
    return nsplit


def _build_bass():
    from concourse import bass, mybir, tile

    f32 = mybir.dt.float32
    dtA, dtX, dtG = _dt(DT_A), _dt(DT_X), _dt(DT_G)
    AF = mybir.ActivationFunctionType

    dt_in = dtA   # x cast to fp16 host-side; stage-1 full-rate fp16
    nc = bass.Bass()
    xd = nc.declare_dram_parameter("x", [NS, 32, 128], dt_in, isOutput=False)
    w1d = nc.declare_dram_parameter("w1d", [64, 128], dt_in, isOutput=False)
    identd = nc.declare_dram_parameter("identd", [128, 128], dtA, isOutput=False)
    w2d = nc.declare_dram_parameter("w2d", [128, 32 * 3 * 64], dtA, isOutput=False)
    wnyqd = nc.declare_dram_parameter("wnyqd", [128, 1], dtA, isOutput=False)
    cmatd = nc.declare_dram_parameter("cmatd", [128, 16 * 128], dtG, isOutput=False)
    smatd = nc.declare_dram_parameter("smatd", [128, 16 * 128], dtG, isOutput=False)
    c0d = nc.declare_dram_parameter("c0d", [128, 16], dtG, isOutput=False)
    cnd = nc.declare_dram_parameter("cnd", [1, 128], dtG, isOutput=False)
    onekd = nc.declare_dram_parameter("onekd", [1, 1], dtG, isOutput=False)

    NBLK = (ROWSH + 127) // 128  # lag-0 col chunks per half
    outpd = nc.declare_dram_parameter("outp", [128, ROWS], f32, isOutput=True)
    outmd = nc.declare_dram_parameter("outm", [128, ROWS], f32, isOutput=True)
    out0d = nc.declare_dram_parameter("out0", [128, HALVES * NBLK], f32,
                                      isOutput=True)

    with tile.TileContext(nc) as tc:
        with (
            tc.tile_pool(name="const", bufs=1) as cpool,
            tc.tile_pool(name="big", bufs=1) as bigp,
        ):
            # --- load constants ---
            w1sb = cpool.tile([64, 128], dt_in, tag="w1sb")
            nc.sync.dma_start(out=w1sb[:], in_=w1d[:])
            identsb = cpool.tile([128, 128], dtA, tag="identsb")
            nc.sync.dma_start(out=identsb[:], in_=identd[:])
            wnyqsb = cpool.tile([128, 1], dtA, tag="wnyqsb")
            nc.sync.dma_start(out=wnyqsb[:], in_=wnyqd[:])
            cmatsb = cpool.tile([128, 2048], dtG, tag="cmatsb")
            nc.sync.dma_start(out=cmatsb[:], in_=cmatd[:])
            smatsb = cpool.tile([128, 2048], dtG, tag="smatsb")
            nc.sync.dma_start(out=smatsb[:], in_=smatd[:])
            c0sb = cpool.tile([128, 16], dtG, tag="c0sb")
            nc.sync.dma_start(out=c0sb[:], in_=c0d[:])
            cnsb = cpool.tile([1, 128], dtG, tag="cnsb")
            nc.sync.dma_start(out=cnsb[:], in_=cnd[:])
            oneksb = cpool.tile([1, 1], dtG, tag="oneksb")
            nc.sync.dma_start(out=oneksb[:], in_=onekd[:])

            Xre = bigp.tile([128, 16 * NS], dtX, tag="Xre")
            Xim = bigp.tile([128, 16 * NS], dtX, tag="Xim")

            xnyqsb = cpool.tile([1, NS], f32, tag="xnyqsb")
            snyq = cpool.tile([1, NS], f32, tag="snyq")
            g2048 = cpool.tile([1, ROWS], dtG, tag="g2048")

            outpsb = cpool.tile([128, ROWS], f32, tag="outpsb")
            outmsb = cpool.tile([128, ROWS], f32, tag="outmsb")
            out0sb = cpool.tile([128, HALVES * NBLK], f32, tag="out0sb")

            # x DRAM view: sig s = 8t + 2j + g -> [t, g, n1, j, n2]
            xview = xd[:].rearrange("(t j g) a b -> t g a j b", t=24, j=4, g=2)

            fwd_scope = tc.tile_pool(name="fwd", bufs=1)
            fwdp = fwd_scope.__enter__()
            xin_scope = tc.tile_pool(name="xin", bufs=4)
            xinp = xin_scope.__enter__()
            s1_scope = tc.tile_pool(name="s1sb", bufs=4)
            s1p = s1_scope.__enter__()

            AT = fwdp.tile([128, 24 * 512], dtA, tag="AT")
            atv = AT[:].rearrange("p (t j g r q) -> p t j g r q",
                                  t=24, j=4, g=2, r=2, q=32)
            w2sb = fwdp.tile([128, 32 * 3 * 64], dtA, tag="w2sb")
            nc.sync.dma_start(out=w2sb[:], in_=w2d[:])

            # ---------- phase A: stage 1 + corner turn ----------
            with tc.tile_pool(name="psA", bufs=3, space="PSUM") as psA:
                for t in range(24):
                    xt = xinp.tile([64, 512], dt_in, tag="xt")
                    nc.sync.dma_start(out=xt[:], in_=xview[t])
                    ps = psA.tile([128, 512], f32, tag="s1")
                    nc.tensor.matmul(ps[:, :], w1sb[:, :], xt[:, :],
                                     start=True, stop=True)
                    s1s = s1p.tile([128, 512], dtA, tag="s1s")
                    if t % 2 == 0:
                        nc.vector.tensor_copy(s1s[:], ps[:])
                    else:
                        nc.scalar.copy(s1s[:], ps[:])
                    tp = psA.tile([128, 512], dtA, tag="tp")
                    for b4 in range(4):
                        nc.tensor.transpose(tp[:, 128 * b4:128 * (b4 + 1)],
                                            s1s[:, 128 * b4:128 * (b4 + 1)],
                                            identsb[:])
                    dst = AT[:, 512 * t:512 * (t + 1)]
                    if t % 2 == 0:
                        nc.scalar.copy(dst, tp[:])
                    else:
                        nc.vector.tensor_copy(dst, tp[:])

                # nyquist: X[2048] = sum_n2 (-1)^n2 * Are[q=0]
                are0 = atv[:, :, :, :, 0, 0]
                psn = psA.tile([1, NS], f32, tag="xnyq", bufs=1)
                nc.tensor.matmul(psn[:], wnyqsb[:], are0, start=True, stop=True)
                nc.scalar.copy(xnyqsb[:], psn[:])

            # ---------- phase B: stage 2 ----------
            w2v = w2sb[:].rearrange("p (q t k) -> p q t k", q=32, t=3, k=64)
            with tc.tile_pool(name="psB", bufs=2, space="PSUM") as psB:
                for jq in range(16):
                    x2 = psB.tile([128, 384], f32, tag="x2")
                    for par in range(2):
                        qv = 2 * jq + par
                        are = atv[:, :, :, :, 0, qv]
                        aim = atv[:, :, :, :, 1, qv]
                        if DT_A == "float32r_never":
                            pass
                        re_out = x2[64 * par:64 * (par + 1), 0:192]
                        im_out = x2[64 * par:64 * (par + 1), 192:384]
                        nc.tensor.matmul(re_out, w2v[:, qv, 0, :], are,
                                         start=True, stop=False)
                        nc.tensor.matmul(re_out, w2v[:, qv, 1, :], aim,
                                         start=False, stop=True)
                        nc.tensor.matmul(im_out, w2v[:, qv, 2, :], are,
                                         start=True, stop=False)
                        nc.tensor.matmul(im_out, w2v[:, qv, 0, :], aim,
                                         start=False, stop=True)
                    # permute (b, n) -> X layout (j, n, b): b innermost for
                    # bf16 2x tensor_tensor in the cross-power stage
                    xrev_ = Xre[:].rearrange("p (j n b) -> p j n b",
                                             j=16, n=NSIG, b=B)
                    ximv_ = Xim[:].rearrange("p (j n b) -> p j n b",
                                             j=16, n=NSIG, b=B)
                    re_in = x2[:, 0:192].rearrange("p (b n) -> p b n", b=B, n=NSIG)
                    im_in = x2[:, 192:384].rearrange("p (b n) -> p b n", b=B, n=NSIG)
                    nc.scalar.copy(xrev_[:, jq].transpose([0, 2, 1]), re_in)
                    nc.scalar.copy(ximv_[:, jq].transpose([0, 2, 1]), im_in)

            # ---------- PHAT (4 j-blocks, pipelined across ACT/DVE) ----------
            t1 = fwdp.tile([128, 16 * NS], f32, tag="t1")
            t2 = fwdp.tile([128, 16 * NS], f32, tag="t2")
            rbf = fwdp.tile([128, 16 * NS], dtX, tag="rbf")
            PB = 4 * NS
            sls = [slice(PB * pb, PB * (pb + 1)) for pb in range(4)]
            for sl in sls:
                nc.scalar.activation(t1[:, sl], Xre[:, sl], AF.Square)
                nc.scalar.activation(t2[:, sl], Xim[:, sl], AF.Square)
                nc.vector.tensor_add(t1[:, sl], t1[:, sl], t2[:, sl])
            for sl in sls:
                nc.scalar.activation(t2[:, sl], t1[:, sl], AF.Ln)
            for sl in sls:
                nc.scalar.activation(rbf[:, sl], t2[:, sl], AF.Exp, scale=-0.5)
            for sl in sls:
                nc.vector.tensor_mul(Xre[:, sl], Xre[:, sl], rbf[:, sl])
                nc.vector.tensor_mul(Xim[:, sl], Xim[:, sl], rbf[:, sl])
            # sign, permuted (b, n) -> (n, b)
            snv = snyq[:].rearrange("p (n b) -> p n b", n=NSIG, b=B)
            nc.scalar.sign(snv[0:1].transpose([0, 2, 1]),
                           xnyqsb[0:1, :].rearrange("p (b n) -> p b n", b=B, n=NSIG))

            # nyquist pair row (layout: (pair, b))
            g2v = g2048[:].rearrange("p (r b) -> p r b", r=NPAIR, b=B)
            for n in range(NSIG - 1):
                mc = NSIG - 1 - n
                an = snv[0:1, n, :].unsqueeze(1).broadcast_to((1, mc, B))
                am = snv[0:1, n + 1:, :]
                nc.vector.tensor_mul(g2v[0:1, POFF[n]:POFF[n] + mc, :], an, am)

            # ---------- cross-power + inverse, per half ----------
            s1_scope.__exit__(None, None, None)
            xin_scope.__exit__(None, None, None)
            fwd_scope.__exit__(None, None, None)
            xrev = Xre[:].rearrange("p (j n b) -> p j n b", j=16, n=NSIG, b=B)
            ximv = Xim[:].rearrange("p (j n b) -> p j n b", j=16, n=NSIG, b=B)
            BH = B // HALVES
            with (
                tc.tile_pool(name="gpool", bufs=1) as gp,
                tc.tile_pool(name="tmpp", bufs=1) as tmpp,
                tc.tile_pool(name="psC", bufs=1, space="PSUM") as psC,
                tc.tile_pool(name="ps0", bufs=2, space="PSUM") as ps0,
            ):
                assert HALVES == 1
                for h in range(HALVES):
                    Gre = gp.tile([128, 16 * ROWSH], dtG, tag="Gre")
                    Gim = gp.tile([128, 16 * ROWSH], dtG, tag="Gim")
                    grev = Gre[:].rearrange("p (j r b) -> p j r b", j=16, r=NPAIR, b=B)
                    gimv = Gim[:].rearrange("p (j r b) -> p j r b", j=16, r=NPAIR, b=B)
                    tt1 = tmpp.tile([128, 16 * B * (NSIG - 1)], dtG, tag="tt1")
                    tt2 = tmpp.tile([128, 16 * B * (NSIG - 1)], dtG, tag="tt2")
                    t1v = tt1[:].rearrange("p (j m b) -> p j m b", j=16, m=NSIG - 1, b=B)
                    t2v = tt2[:].rearrange("p (j m b) -> p j m b", j=16, m=NSIG - 1, b=B)
                    grev2 = Gre[:].rearrange("p (j r) -> p j r", j=16, r=ROWSH)
                    gimv2 = Gim[:].rearrange("p (j r) -> p j r", j=16, r=ROWSH)
                    rcs = []
                    rc0 = 0
                    while rc0 < ROWSH:
                        rcs.append((rc0, min(512, ROWSH - rc0)))
                        rc0 += 512
                    eps_t, ops_t = {}, {}
                    for jh in range(2):
                        js = slice(8 * jh, 8 * (jh + 1))
                        for n in range(NSIG - 1):
                            mc = NSIG - 1 - n
                            an = xrev[:, js, n, :].unsqueeze(2).broadcast_to(
                                (128, 8, mc, B))
                            bn = ximv[:, js, n, :].unsqueeze(2).broadcast_to(
                                (128, 8, mc, B))
                            am = xrev[:, js, n + 1:, :]
                            bm = ximv[:, js, n + 1:, :]
                            o_re = grev[:, js, POFF[n]:POFF[n] + mc, :]
                            o_im = gimv[:, js, POFF[n]:POFF[n] + mc, :]
                            u1 = t1v[:, js, 0:mc, :]
                            u2 = t2v[:, js, 0:mc, :]
                            nc.vector.tensor_mul(u1, an, am)
                            nc.vector.tensor_mul(u2, bn, bm)
                            nc.vector.tensor_add(o_re, u1, u2)
                            nc.vector.tensor_mul(u1, bn, am)
                            nc.vector.tensor_mul(u2, an, bm)
                            nc.vector.tensor_sub(o_im, u1, u2)
                        # inverse accumulation for this j-half (overlaps the
                        # other half's cross-power on DVE)
                        for ci, (c0, cw) in enumerate(rcs):
                            if jh == 0:
                                eps_t[ci] = psC.tile([128, cw], f32, tag=f"e{ci}", name=f"eps{ci}")
                                ops_t[ci] = psC.tile([128, cw], f32, tag=f"o{ci}", name=f"ops{ci}")
                            eps, ops_ = eps_t[ci], ops_t[ci]
                            for jq in range(8 * jh, 8 * jh + 8):
                                nc.tensor.matmul(
                                    eps[:], cmatsb[:, 128 * jq:128 * (jq + 1)],
                                    grev2[:, jq, c0:c0 + cw],
                                    start=(jq == 0), stop=False)
                            for jq in range(8 * jh, 8 * jh + 8):
                                nc.tensor.matmul(
                                    ops_[:], smatsb[:, 128 * jq:128 * (jq + 1)],
                                    gimv2[:, jq, c0:c0 + cw],
                                    start=(jq == 0), stop=(jq == 15))
                    for ci, (c0, cw) in enumerate(rcs):
                        gc = h * ROWSH + c0
                        eps, ops_ = eps_t[ci], ops_t[ci]
                        nc.tensor.matmul(eps[:], cnsb[:], g2048[:, gc:gc + cw],
                                         start=False, stop=True)
                        esb = tmpp.tile([128, 512], f32, tag="esb")
                        nc.scalar.copy(esb[0:128, 0:cw], eps[:])
                        nc.vector.tensor_add(outpsb[:, gc:gc + cw],
                                             esb[0:128, 0:cw], ops_[:])
                        nc.vector.tensor_sub(outmsb[:, gc:gc + cw],
                                             esb[0:128, 0:cw], ops_[:])
                        nc.sync.dma_start(out=outpd[:, gc:gc + cw],
                                          in_=outpsb[:, gc:gc + cw])
                        nc.sync.dma_start(out=outmd[:, gc:gc + cw],
                                          in_=outmsb[:, gc:gc + cw])

                    # lag-0 side channel
                    for bi in range(NBLK):
                        c0 = 128 * bi
                        cw = min(128, ROWSH - c0)
                        gc = h * ROWSH + c0
                        z = ps0.tile([128, 1], f32, tag="z")
                        for jq in range(16):
                            nc.tensor.matmul(z[0:cw, :], grev2[:, jq, c0:c0 + cw],
                                             c0sb[:, jq:jq + 1],
                                             start=(jq == 0), stop=False)
                        nc.tensor.matmul(z[0:cw, :], g2048[:, gc:gc + cw],
                                         oneksb[:], start=False, stop=True)
                        nc.scalar.copy(out0sb[0:cw, h * NBLK + bi:h * NBLK + bi + 1],
                                       z[0:cw, :])
            nc.sync.dma_start(out=out0d[:], in_=out0sb[:])

    _legalize_waits(nc)
    return nc


def _get_compiled():
    if "nc" not in _COMPILED:
        _COMPILED["nc"] = _build_bass()
        _COMPILED["weights"] = _build_weights()
    return _COMPILED["nc"], _COMPILED["weights"]


def kernel(x: np.ndarray) -> np.ndarray:
    from concourse.bass_utils import run_bass_kernel_spmd

    nc, W = _get_compiled()
    x = np.ascontiguousarray(x, dtype=np.float32)
    xdev = x.astype(_npdt(DT_A))
    in_maps = []
    for c in range(NCORES):
        xc = xdev[c * B:(c + 1) * B].reshape(NS, 32, 128)
        m = {"x": np.ascontiguousarray(xc)}
        m.update(W)
        in_maps.append(m)

    trace = bool(int(os.environ.get("BASS_GCC_TRACE", "0")))
    res = run_bass_kernel_spmd(nc, in_maps, list(range(NCORES)), trace=trace)
    _COMPILED["last_result"] = res

    NBLK = (ROWSH + 127) // 128
    out = np.zeros((NCORES * B, NSIG, NSIG, 2 * TAU + 1), dtype=np.float32)
    for c in range(NCORES):
        r = res.results[c]
        outp, outm, out0 = r["outp"], r["outm"], r["out0"]
        # lag-0 per row (row index = pair*B + b)
        z = np.zeros(ROWS, np.float32)
        for bi in range(NBLK):
            c0 = 128 * bi
            cw = min(128, ROWS - c0)
            z[c0:c0 + cw] = out0[0:cw, bi]
        for pi, (n, m) in enumerate(PAIRS):
            rows = pi * B + np.arange(B)
            blk = out[c * B:(c + 1) * B]
            blk[:, n, m, 0] = z[rows]
            blk[:, n, m, 1:129] = outp[:, rows].T
            blk[:, n, m, 129:] = outm[::-1, rows].T
            blk[:, m, n, 0] = z[rows]
            blk[:, m, n, 1:] = blk[:, n, m, 1:][:, ::-1]
        for n in range(NSIG):
            out[c * B:(c + 1) * B, n, n, 0] = 1.0
    return out
